# revision 21
# baseline (speedup 1.0000x reference)
"""ActionVQVAE forward kernel for 8x Trainium2 NeuronCores.

Data parallel: x [262144, 28] sharded along batch across 8 cores; tiny
MLP weights / codebook replicated.

Device work per core (32768 rows): encoder GEMMs (fp32r), negated
distance matmul d' = 2 z.e - |e|^2 (argmax d' == argmin dist), DVE
max8 + max_index scan -> per-row best-code indices, plus partial sums
for the losses (sum of row maxima, sum z^2).

Host work: decoder folded into a 512x28 table (decoder input takes only
512 values); recon = table[idx]; scalar losses assembled from per-core
sums.

Self-contained: hardcodes all shapes from the problem spec.
"""

import sys

sys.path.insert(0, "/opt/trn_rl_repo")

import numpy as np

import concourse.bacc as bacc
import concourse.mybir as mybir
import concourse.tile as tile

F32 = mybir.dt.float32
F32R = mybir.dt.float32r
BF16 = mybir.dt.bfloat16
U32 = mybir.dt.uint32

N_CORES = 8
B_TOTAL = 262144
B_CORE = B_TOTAL // N_CORES          # 32768
IN_DIM = 28
HID = 128
LAT = 64
NCODES = 512
CHUNK = 512                          # batch rows per chunk
N_CHUNKS = B_CORE // CHUNK           # 64
NPC = CHUNK // 128                   # 4 sub-tiles of 128 rows per chunk
COMMITMENT_COST = 0.25

_CACHE = {}


def build_program(n_chunks=N_CHUNKS):
    nc = bacc.Bacc("TRN2", target_bir_lowering=False, debug=False)

    b_core = n_chunks * CHUNK
    x_in = nc.dram_tensor("x", [b_core, IN_DIM], F32R, kind="ExternalInput")
    w1_in = nc.dram_tensor("w1", [IN_DIM, HID], F32R, kind="ExternalInput")
    b1_in = nc.dram_tensor("b1", [HID, 1], F32, kind="ExternalInput")
    w2_in = nc.dram_tensor("w2", [HID, LAT], F32R, kind="ExternalInput")
    b2_in = nc.dram_tensor("b2", [LAT, 1], F32, kind="ExternalInput")
    rhsa_in = nc.dram_tensor("rhsa", [LAT + 1, NCODES], F32R, kind="ExternalInput")
    onesr_in = nc.dram_tensor("onesr", [1, CHUNK], F32R, kind="ExternalInput")
    ident_in = nc.dram_tensor("ident", [128, 128], F32R, kind="ExternalInput")
    idx_out = nc.dram_tensor("idxo", [n_chunks, 128, NPC * 8], U32,
                             kind="ExternalOutput")
    sums_out = nc.dram_tensor("sums", [2, 1], F32, kind="ExternalOutput")

    with tile.TileContext(nc) as tc:
        with (
            tc.tile_pool(name="const", bufs=1) as cpool,
            tc.tile_pool(name="xnat", bufs=n_chunks) as xnat_pool,
            tc.tile_pool(name="store", bufs=1) as store_pool,
            tc.tile_pool(name="xt", bufs=3) as xt_pool,
            tc.tile_pool(name="h", bufs=3) as h_pool,
            tc.tile_pool(name="z", bufs=3) as z_pool,
            tc.tile_pool(name="dist", bufs=8) as dist_pool,
            tc.tile_pool(name="idx", bufs=3) as idx_pool,
            tc.tile_pool(name="scratch", bufs=2) as scratch_pool,
            tc.tile_pool(name="ps_xt", bufs=1, space="PSUM") as ps_xt,
            tc.tile_pool(name="ps_h", bufs=2, space="PSUM") as ps_h,
            tc.tile_pool(name="ps_z", bufs=1, space="PSUM") as ps_z,
            tc.tile_pool(name="ps_dist", bufs=3, space="PSUM") as ps_dist,
        ):
            # ---- constants -------------------------------------------------
            w1 = cpool.tile([IN_DIM, HID], F32R, tag="w1")
            nc.sync.dma_start(w1[:], w1_in[:, :])
            b1 = cpool.tile([HID, 1], F32, tag="b1")
            nc.sync.dma_start(b1[:], b1_in[:, :])
            w2 = cpool.tile([HID, LAT], F32R, tag="w2")
            nc.sync.dma_start(w2[:], w2_in[:, :])
            b2 = cpool.tile([LAT, 1], F32, tag="b2")
            nc.sync.dma_start(b2[:], b2_in[:, :])
            rhsa = cpool.tile([LAT + 1, NCODES], F32R, tag="rhsa")
            nc.sync.dma_start(rhsa[:], rhsa_in[:, :])
            onesr = cpool.tile([1, CHUNK], F32R, tag="onesr")
            nc.sync.dma_start(onesr[:], onesr_in[:, :])
            ident = cpool.tile([128, 128], F32R, tag="ident")
            nc.sync.dma_start(ident[:], ident_in[:, :])
            ones_col = cpool.tile([128, 1], F32, tag="ones_col")
            nc.vector.memset(ones_col[:], 1.0)

            # persistent stores for partial sums
            m_store = store_pool.tile([128, n_chunks * NPC * 8], F32, tag="m_store")
            z2_store = store_pool.tile([LAT, n_chunks], F32, tag="z2_store")

            xt_tiles = [
                xt_pool.tile([IN_DIM, CHUNK], F32R, tag="xt", name=f"xt{i}")
                for i in range(3)
            ]
            h_tiles = [
                h_pool.tile([HID, CHUNK], F32R, tag="h", name=f"h{i}")
                for i in range(3)
            ]
            z_tiles = []
            for i in range(3):
                t = z_pool.tile([LAT + 1, CHUNK], F32R, tag="z", name=f"z{i}")
                nc.sync.dma_start(t[LAT : LAT + 1, :], onesr[:])
                z_tiles.append(t)
            dist_tiles = [
                dist_pool.tile([128, NCODES], F32, tag="dist", name=f"dist{i}")
                for i in range(8)
            ]
            idx_tiles = [
                idx_pool.tile([128, NPC * 8], U32, tag="idx", name=f"idx{i}")
                for i in range(3)
            ]
            zsq_tiles = [
                scratch_pool.tile([LAT, CHUNK], BF16, tag="zsq", name=f"zsq{i}")
                for i in range(2)
            ]
            xnat_tiles = [
                xnat_pool.tile([128, NPC * IN_DIM], F32R, tag="xnat", name=f"xnat{i}")
                for i in range(n_chunks)
            ]

            # ---- main loop -------------------------------------------------
            for c in range(n_chunks):
                xnat = xnat_tiles[c]
                xt = xt_tiles[c % 3]
                h = h_tiles[c % 3]
                z = z_tiles[c % 3]
                idx = idx_tiles[c % 3]
                zsq = zsq_tiles[c % 2]

                # load x chunk: partition p <- rows c*512 + p*4 + n
                src = x_in[c * CHUNK : (c + 1) * CHUNK, :].rearrange(
                    "(p n) k -> p n k", p=128
                )
                nc.sync.dma_start(xnat[:].rearrange("p (n k) -> p n k", n=NPC), src)

                # transpose to x^T
                xt_ps = ps_xt.tile([IN_DIM, CHUNK], F32, tag="xt_ps", name=f"xtps{c}")
                for j in range(NPC):
                    nc.tensor.transpose(
                        xt_ps[:, j * 128 : (j + 1) * 128].bitcast(F32R),
                        xnat[:, j * IN_DIM : (j + 1) * IN_DIM],
                        ident[:],
                    )
                nc.scalar.activation(
                    xt[:], xt_ps[:], mybir.ActivationFunctionType.Copy
                )

                # encoder GEMM1 + relu(+b1)
                h_ps = ps_h.tile([HID, CHUNK], F32, tag="h_ps", name=f"hps{c}")
                nc.tensor.matmul(h_ps[:], w1[:], xt[:], start=True, stop=True)
                nc.scalar.activation(
                    h[:], h_ps[:], mybir.ActivationFunctionType.Relu, bias=b1[:]
                )

                # encoder GEMM2 (+b2 via ACT bias)
                z_ps = ps_z.tile([LAT, CHUNK], F32, tag="z_ps", name=f"zps{c}")
                nc.tensor.matmul(z_ps[:], w2[:], h[:], start=True, stop=True)
                nc.scalar.activation(
                    z[0:LAT, :],
                    z_ps[:],
                    mybir.ActivationFunctionType.Identity,
                    bias=b2[:],
                )
                # sum((z+b2)^2) for this chunk via ACT accumulator
                nc.scalar.activation(
                    zsq[:],
                    z_ps[:],
                    mybir.ActivationFunctionType.Square,
                    bias=b2[:],
                    accum_out=z2_store[:, c : c + 1],
                )

                # negated distances (2 z.e - e^2) + argmax scan
                for j in range(NPC):
                    col = (c * NPC + j) * 8
                    d_ps = ps_dist.tile(
                        [128, NCODES], F32, tag="d_ps", name=f"dps{c}_{j}"
                    )
                    nc.tensor.matmul(
                        d_ps[:],
                        z[:, j * 128 : (j + 1) * 128],
                        rhsa[:],
                        start=True,
                        stop=True,
                    )
                    m8 = m_store[:, col : col + 8]
                    nc.vector.max(m8, d_ps[:])
                    dist_sb = dist_tiles[(c * NPC + j) % 8]
                    nc.scalar.activation(
                        dist_sb[:], d_ps[:], mybir.ActivationFunctionType.Copy
                    )
                    nc.vector.max_index(
                        idx[:, j * 8 : (j + 1) * 8], m8, dist_sb[:]
                    )

                nc.sync.dma_start(idx_out[c, :, :], idx[:])

            # ---- final cross-partition reduction --------------------------
            mred = store_pool.tile([128, 1], F32, tag="mred")
            nc.vector.reduce_sum(
                mred[:],
                m_store[:, 0 : n_chunks * NPC * 8 : 8],
                axis=mybir.AxisListType.X,
            )
            z2red = store_pool.tile([LAT, 1], F32, tag="z2red")
            nc.vector.reduce_sum(z2red[:], z2_store[:], axis=mybir.AxisListType.X)

            packed = store_pool.tile([128, 2], F32, tag="packed")
            nc.vector.memset(packed[:], 0.0)
            nc.vector.tensor_copy(packed[:, 0:1], mred[:])
            nc.vector.tensor_copy(packed[0:LAT, 1:2], z2red[:])

            sums_ps = ps_z.tile([2, 1], F32, tag="sums_ps")
            nc.tensor.matmul(
                sums_ps[:], packed[:], ones_col[:], start=True, stop=True
            )
            sums_sb = store_pool.tile([2, 1], F32, tag="sums_sb")
            nc.vector.tensor_copy(sums_sb[:], sums_ps[:])
            nc.sync.dma_start(sums_out[:, :], sums_sb[:])

    nc.compile()
    return nc


def _host_prep(enc_w1, enc_b1, enc_w2, enc_b2, codebook, dec_w1, dec_b1, dec_w2,
               dec_b2):
    w1 = np.ascontiguousarray(enc_w1, dtype=np.float32)
    b1 = np.ascontiguousarray(enc_b1, dtype=np.float32).reshape(HID, 1)
    w2 = np.ascontiguousarray(enc_w2, dtype=np.float32)
    b2 = np.ascontiguousarray(enc_b2, dtype=np.float32).reshape(LAT, 1)
    # negated distance operand: d' = 2 z.e - ||e||^2 (argmax d' == argmin dist)
    e2 = (codebook.astype(np.float64) ** 2).sum(axis=1)
    rhsa = np.concatenate(
        [2.0 * codebook.T.astype(np.float64), -e2[None, :]], axis=0
    ).astype(np.float32)
    onesr = np.ones((1, CHUNK), dtype=np.float32)
    # decoder table: recon row for each code (fp32, matching reference math)
    hd = np.maximum(codebook @ dec_w1 + dec_b1, 0.0).astype(np.float32)
    table = (hd @ dec_w2 + dec_b2).astype(np.float32)
    ident = np.eye(128, dtype=np.float32)
    return w1, b1, w2, b2, rhsa, onesr, table, ident


def _unpermute_idx(idxo, n_chunks=N_CHUNKS):
    """idxo [n_chunks, 128, NPC*8] uint32 -> flat [n_chunks*CHUNK] indices.

    Batch row c*CHUNK + p*NPC + j lives at idxo[c, p, j*8].
    """
    sel = idxo[:, :, 0 : NPC * 8 : 8]          # [n_chunks, 128, NPC]
    return sel.reshape(-1).astype(np.int64)


def kernel(x, enc_w1, enc_b1, enc_w2, enc_b2, codebook, dec_w1, dec_b1, dec_w2,
           dec_b2):
    from concourse.bass_utils import run_bass_kernel_spmd

    w1, b1, w2, b2, rhsa, onesr, table, ident = _host_prep(
        enc_w1, enc_b1, enc_w2, enc_b2, codebook, dec_w1, dec_b1, dec_w2, dec_b2
    )

    if "nc" not in _CACHE:
        _CACHE["nc"] = build_program()
    nc = _CACHE["nc"]

    x = np.ascontiguousarray(x, dtype=np.float32)
    in_maps = []
    for i in range(N_CORES):
        in_maps.append(
            {
                "x": x[i * B_CORE : (i + 1) * B_CORE],
                "w1": w1, "b1": b1,
                "w2": w2, "b2": b2,
                "rhsa": rhsa, "onesr": onesr, "ident": ident,
            }
        )

    res = run_bass_kernel_spmd(nc, in_maps, core_ids=list(range(N_CORES)))

    idx_all = np.concatenate(
        [_unpermute_idx(res.results[i]["idxo"]) for i in range(N_CORES)]
    )
    sums = np.stack([res.results[i]["sums"][:, 0] for i in range(N_CORES)])
    s_m = float(sums[:, 0].astype(np.float64).sum())
    s_z2 = float(sums[:, 1].astype(np.float64).sum())

    recon = table[idx_all]
    diff = recon - x
    s_r2 = float(np.einsum("ij,ij->", diff, diff, dtype=np.float64))

    recon_loss = np.float32(s_r2 / (B_TOTAL * IN_DIM))
    # sum of min distances = sum(z^2) - sum(max(2 z.e - e^2))
    vq_loss = np.float32((1.0 + COMMITMENT_COST) * (s_z2 - s_m) / (B_TOTAL * LAT))
    return recon, recon_loss, vq_loss
